# revision 18
# baseline (speedup 1.0000x reference)
"""MixedScoreMultiHeadAttention Trainium2 kernel, v2.

Data-parallel over batch: 32 batches -> 8 cores x 4 batches.

Cost-model-driven design (TimelineSim):
 - matmul cost = out-free-size x 0.417ns (fp16 moving); ldweights free
 - ACT/DVE evac cost = free-size x 0.83/1.04 ns + ~0.13-0.37us fixed/op
 - Pool cannot read PSUM; evac of the mix-MLP hidden layer (131072
   rows/core) across ACT+DVE is the bottleneck -> use 2-bank [128,1024]
   PSUM tiles so each evac op moves 1024 rows, and balance ACT/DVE.
 - full-partition single-op projection evacs (zero-padded head slots ride
   along free); one-DMA rhs assembly per batch (HWDGE is 625ns/DMA).

Pipeline per (b): dots (PE) -> d_all fp16 -> DMA-assemble rhs [17, 16384]
(r-major points; row 16 = cost) -> layer1 [17x128 stationary] per half ->
relu evac (bias folded, ACT/DVE alternating) -> layer2 (stationary = data,
8-col moving w2 blockdiag) -> exp evac -> AV with ones-column denominator
-> reciprocal * numerator.

mix2 bias dropped (softmax-invariant); 1/sqrt(D) folded into Wq host-side.
"""
import sys

sys.path.insert(0, "/opt/trn_rl_repo")

import numpy as np
from contextlib import ExitStack

import concourse.bass as bass
import concourse.mybir as mybir
import concourse.tile as tile
from concourse import bacc
from concourse.bass_utils import run_bass_kernel_spmd
from concourse.masks import make_identity

B, R, C, E, H, D, MS = 32, 128, 128, 256, 16, 16, 16
NCORES = 8
BL = B // NCORES  # batches per core: 4
TOK = BL * R      # 512 tokens per core per side
PTS = R * C       # 16384 score points per (b)

FP32 = mybir.dt.float32
FP16 = mybir.dt.float16
AF = mybir.ActivationFunctionType
ALU = mybir.AluOpType


def cp(nc, use_act, out, in_):
    if use_act:
        nc.scalar.copy(out, in_)
    else:
        nc.vector.tensor_copy(out, in_)


import os
STAGE = int(os.environ.get("V2_STAGE", "50"))


def build_kernel():
    nc = bacc.Bacc("TRN2", target_bir_lowering=False, debug=False,
                   num_devices=NCORES)

    x_r = nc.dram_tensor("x_r", [TOK, E], FP32, kind="ExternalInput").ap()
    x_c = nc.dram_tensor("x_c", [TOK, E], FP32, kind="ExternalInput").ap()
    cost = nc.dram_tensor("cost", [BL, R, C], FP32, kind="ExternalInput").ap()
    wq_d = nc.dram_tensor("Wq", [E, E], FP32, kind="ExternalInput").ap()
    wk_d = nc.dram_tensor("Wk", [E, E], FP32, kind="ExternalInput").ap()
    wv_d = nc.dram_tensor("Wv", [E, E], FP32, kind="ExternalInput").ap()
    # layer1 stationary [17, 256]: col (half*128 + (h%8)*16 + m):
    #   row h' = a[h,m] iff h'==h; row 16 = c[h,m]
    w1_d = nc.dram_tensor("W1L", [17, 2 * 128], FP32,
                          kind="ExternalInput").ap()
    # layer2 moving [128, 16]: col (half*8 + j): row hm = w2[half*8+j, m]
    # iff hm == (j*16+m) else 0
    w2_d = nc.dram_tensor("W2L", [128, 16], FP32, kind="ExternalInput").ap()
    # relu bias per (h,m) row: bcol2[hm, half] = b1[half*8 + hm//16, hm%16]
    bc_d = nc.dram_tensor("bcol2", [128, 2], FP32, kind="ExternalInput").ap()
    out_d = nc.dram_tensor("out", [BL, R, H * D], FP32,
                           kind="ExternalOutput").ap()

    with tile.TileContext(nc) as tc, ExitStack() as ctx:
        const_p = ctx.enter_context(tc.tile_pool(name="const", bufs=1))
        inx_p = ctx.enter_context(tc.tile_pool(name="inx", bufs=2))
        qkv_p = ctx.enter_context(tc.tile_pool(name="qkv", bufs=1))
        dall_p = ctx.enter_context(tc.tile_pool(name="dall", bufs=2))
        rhs_p = ctx.enter_context(tc.tile_pool(name="rhs", bufs=2))
        rr_p = ctx.enter_context(tc.tile_pool(name="rr", bufs=3))
        wsb_p = ctx.enter_context(tc.tile_pool(name="wsb", bufs=2))
        fout_p = ctx.enter_context(tc.tile_pool(name="fout", bufs=1))
        small_p = ctx.enter_context(tc.tile_pool(name="small", bufs=2))
        # PSUM: pl1 2x2 banks + ps2big 3 banks + dt 1 bank = 8 banks
        pl1_p = ctx.enter_context(
            tc.tile_pool(name="pl1", bufs=2, space="PSUM"))
        ps2_p = ctx.enter_context(
            tc.tile_pool(name="ps2", bufs=1, space="PSUM"))
        psa_p = ctx.enter_context(
            tc.tile_pool(name="psa", bufs=1, space="PSUM"))
        dt_p = ctx.enter_context(
            tc.tile_pool(name="dt", bufs=1, space="PSUM"))

        ident = const_p.tile([128, 128], FP32)
        make_identity(nc, ident[:])

        # ---- small weight/const loads
        w1f = inx_p.tile([17, 2 * 128], FP32, tag="w1f")
        nc.sync.dma_start(w1f[:], w1_d[:])
        w1l = const_p.tile([17, 2 * 128], FP16)
        nc.vector.tensor_copy(w1l[:], w1f[:])

        w2f = inx_p.tile([128, 16], FP32, tag="w2f")
        nc.sync.dma_start(w2f[:], w2_d[:])
        w2l = const_p.tile([128, 16], FP16)
        nc.vector.tensor_copy(w2l[:], w2f[:])

        bcol2 = const_p.tile([128, 2], FP32)
        nc.sync.dma_start(bcol2[:], bc_d[:])

        # ---- QKV weights fp16 (q/k padded on-chip: head h -> 32-col slot)
        wt16 = {}
        for name, dram in (("q", wq_d), ("k", wk_d), ("v", wv_d)):
            w32 = inx_p.tile([128, 2 * E], FP32, tag=f"wload{name}",
                             name=f"wload{name}")
            nc.sync.dma_start(
                w32[:].rearrange("p (s e) -> p s e", s=2),
                dram[:].rearrange("(s p) e -> p s e", p=128))
            halves = []
            for eh in range(2):
                ncols = E if name == "v" else 2 * E
                w16 = const_p.tile([128, ncols], FP16, tag=f"w16{name}{eh}",
                                   name=f"w16{name}{eh}")
                w32e = w32[:, eh * E:(eh + 1) * E]
                if name == "v":
                    nc.scalar.copy(w16[:], w32e)
                else:
                    nc.gpsimd.memset(w16[:], 0.0)
                    w16v = w16[:].rearrange("p (h x) -> p h x", h=H)
                    w32v = w32e.rearrange("p (h x) -> p h x", h=H)
                    nc.vector.tensor_copy(w16v[:, :, 0:D], w32v[:])
                halves.append(w16)
            wt16[name] = halves

        # ---- x load + PE transpose -> xT fp16 [2 e-halves][128, TOK]
        # transposes packed into dt-pool [128,512] as (eh, t%2) quadrants,
        # evac'd 2 cols-of-128 at a time per eh.
        xT = {}
        for name, dram in (("r", x_r), ("c", x_c)):
            xt0 = const_p.tile([128, TOK], FP16, tag=f"xT{name}0",
                               name=f"xT{name}0")
            xt1 = const_p.tile([128, TOK], FP16, tag=f"xT{name}1",
                               name=f"xT{name}1")
            xT[name] = [xt0, xt1]
            xld = inx_p.tile([128, 4 * E], FP32, tag=f"xload{name}",
                             name=f"xload{name}")
            nc.sync.dma_start(
                xld[:].rearrange("p (t e) -> p t e", t=4),
                dram[:].rearrange("(t p) e -> p t e", p=128))
            for tp in range(2):  # token-tile pairs (2t, 2t+1)
                pst = dt_p.tile([128, 512], FP32, tag="dt")
                for ti in range(2):
                    t = tp * 2 + ti
                    for eh in range(2):
                        nc.tensor.transpose(
                            pst[:, eh * 256 + ti * 128:
                                eh * 256 + ti * 128 + 128],
                            xld[:, t * E + eh * 128:t * E + eh * 128 + 128],
                            ident[:])
                for eh in range(2):
                    cp(nc, (tp + eh) % 2 == 0,
                       xT[name][eh][:, tp * 256:(tp + 1) * 256],
                       pst[:, eh * 256:eh * 256 + 256])

        # ---- cost -> y16_all fp16 [128 r, (b, c)]
        y32 = inx_p.tile([128, BL * C], FP32, tag="y32")
        nc.sync.dma_start(
            y32[:].rearrange("p (b c) -> p b c", b=BL),
            cost[:].rearrange("b r c -> r b c"))
        y16 = const_p.tile([128, BL * C], FP16)
        nc.vector.tensor_copy(y16[:], y32[:])

        # ---- projections q/k: full-partition quad tiles [128, 2048] fp16
        # (head h lives at partition slot (h%4)*32, mh = h//4 selects the
        #  512-col token range; zero rows ride along for free)
        qkT = {}
        qk3 = {}
        for proj in ("q", "k"):
            qt = const_p.tile([128, 4 * TOK], FP16, tag=f"qkT{proj}",
                              name=f"qkT{proj}")
            qkT[proj] = qt
            src = xT["r"] if proj == "q" else xT["c"]
            for mp in range(2):  # mh pairs
                ps = pl1_p.tile([128, 1024], FP32, tag="l1")
                for mi in range(2):
                    mh = mp * 2 + mi
                    for eh in range(2):
                        nc.tensor.matmul(
                            ps[:, mi * 512:(mi + 1) * 512],
                            wt16[proj][eh][:, mh * 128:(mh + 1) * 128],
                            src[eh][:], start=(eh == 0), stop=(eh == 1))
                cp(nc, mp % 2 == 0,
                   qt[:, mp * 1024:(mp + 1) * 1024], ps[:])
            # non-base-0 matmul operands crash walrus-lowered HW here --
            # re-home head slots 1..3 to base-0 tiles (cheap SBUF fp16)
            qk3[proj] = {}
            for cls in (1, 2, 3):
                q3 = const_p.tile([16, 4 * TOK], FP16,
                                  tag=f"qk3{proj}{cls}",
                                  name=f"qk3{proj}{cls}")
                qk3[proj][cls] = q3
                nc.vector.tensor_copy(q3[:], qt[cls * 32:cls * 32 + 16, :])

        def qk_slice(proj, h, b):
            # [16, 128] operand, always at base partition 0
            mh, cls = h // 4, h % 4
            lo = mh * TOK + b * 128
            if cls == 0:
                return qkT[proj][0:16, lo:lo + 128]
            return qk3[proj][cls][0:16, lo:lo + 128]

        # ---- v -> vhat_all [128 c, (b, h, 17)] fp32, ones in col 16
        vhat = qkv_p.tile([128, BL * H * 17], FP32)
        vh4 = vhat[:].rearrange("p (b h x) -> p b h x", b=BL, h=H)
        nc.gpsimd.memset(vh4[:, :, :, 16:17], 1.0)
        psv = pl1_p.tile([128, 1024], FP32, tag="l1")
        for b4 in range(BL):
            for eh in range(2):
                nc.tensor.matmul(
                    psv[:, b4 * 256:b4 * 256 + 256],
                    xT["c"][eh][:, b4 * 128:(b4 + 1) * 128],
                    wt16["v"][eh][:], start=(eh == 0), stop=(eh == 1))
        nc.scalar.copy(
            vh4[:, :, :, 0:16],
            psv[:].rearrange("p (b h x) -> p b h x", b=BL, h=H))

        # ---- per-b pipeline
        fouts = [fout_p.tile([128, H * D], FP32, tag=f"fo{b}",
                             name=f"fo{b}") for b in range(BL)]
        for fo in fouts:
            nc.gpsimd.memset(fo[:], 0.0)
        for b in range(BL if STAGE >= 20 else 0):
            # dots: 4 rounds of 4 heads -> d_all [128 r, (h, c)] fp16
            d_all = dall_p.tile([128, H * C], FP16, tag="dall")
            hsel = {20: (0, 1, 2, 3), 21: (0,), 22: (0, 3), 23: (0, 1),
                    24: (0, 2)}.get(STAGE, (0, 1, 2, 3))
            for r4 in range(4):
                psd = dt_p.tile([128, 512], FP32, tag="dt")
                for hh in range(4):
                    h = r4 * 4 + hh
                    if hh not in hsel:
                        continue
                    nc.tensor.matmul(
                        psd[:, hh * 128:(hh + 1) * 128],
                        qk_slice("q", h, b), qk_slice("k", h, b))
                cp(nc, r4 % 2 == 0,
                   d_all[:, r4 * 512:(r4 + 1) * 512], psd[:])

            # rhs assembly: [17, PTS] r-major points (one DMA per head:
            # SBUF APs are partition-first, so the h->partition reshuffle
            # needs per-row DMAs)
            rhs = rhs_p.tile([17, PTS], FP16, tag="rhs")
            for h in range(H if STAGE >= 25 else 0):
                nc.sync.dma_start(rhs[h:h + 1, :],
                                  d_all[:, h * C:(h + 1) * C])
            if STAGE >= 25:
                nc.sync.dma_start(rhs[16:17, :],
                                  y16[:, b * C:(b + 1) * C])

            for half in range(2 if STAGE >= 30 else 0):
                ps2 = ps2_p.tile([128, 1024], FP32, tag="ps2big")
                for ck in range(16):  # 1024-pt chunks
                    pl = pl1_p.tile([128, 1024], FP32, tag="l1")
                    for s2 in range(2):
                        nc.tensor.matmul(
                            pl[:, s2 * 512:(s2 + 1) * 512],
                            w1l[:, half * 128:(half + 1) * 128],
                            rhs[:, ck * 1024 + s2 * 512:
                                ck * 1024 + (s2 + 1) * 512])
                    rr = rr_p.tile([128, 1024], FP16, tag="rr")
                    if ck % 2 == 0:
                        nc.scalar.activation(
                            rr[:], pl[:], AF.Relu,
                            bias=bcol2[:, half:half + 1])
                    else:
                        nc.vector.tensor_scalar(
                            rr[:], pl[:], bcol2[:, half:half + 1],
                            0.0, ALU.add, ALU.max)
                    for s in range(8 if STAGE >= 40 else 0):
                        rloc = ck * 8 + s
                        nc.tensor.matmul(
                            ps2[:, rloc * 8:rloc * 8 + 8],
                            rr[:, s * 128:(s + 1) * 128],
                            w2l[:, half * 8:(half + 1) * 8])
                if STAGE < 50:
                    continue
                # exp evac: [c, (r, h)] fp32
                wsb = wsb_p.tile([128, 8 * C], FP32, tag="wsb")
                nc.scalar.activation(wsb[:], ps2[:], AF.Exp)

                # AV + normalize for the 8 heads of this half
                psa = psa_p.tile([128, 8 * 17], FP32, tag="psa")
                wsb3 = wsb[:].rearrange("p (r h) -> p r h", r=R)
                for hl in range(8):
                    h = half * 8 + hl
                    nc.tensor.matmul(
                        psa[:, hl * 17:(hl + 1) * 17],
                        wsb3[:, :, hl],
                        vh4[:, b, h, :])
                rec = small_p.tile([128, 8], FP32, tag="rec")
                psa3 = psa[:].rearrange("p (x y) -> p x y", x=8)
                nc.vector.reciprocal(rec[:], psa3[:, :, 16])
                nc.vector.tensor_tensor(
                    fouts[b][:, half * 128:(half + 1) * 128].rearrange(
                        "p (x y) -> p x y", x=8),
                    psa3[:, :, 0:16],
                    rec[:].unsqueeze(-1).broadcast_to([128, 8, 16]),
                    ALU.mult)

            nc.sync.dma_start(out_d[b], fouts[b][:])

    nc.compile()
    return nc


_cache = {}


def kernel(**inputs):
    row_emb = np.asarray(inputs["row_emb"], dtype=np.float32)
    col_emb = np.asarray(inputs["col_emb"], dtype=np.float32)
    cost_mat = np.asarray(inputs["cost_mat"], dtype=np.float32)
    Wq = np.asarray(inputs["Wq"], dtype=np.float32)
    Wk = np.asarray(inputs["Wk"], dtype=np.float32)
    Wv = np.asarray(inputs["Wv"], dtype=np.float32)
    m1w = np.asarray(inputs["mix1_weight"], dtype=np.float32)
    m1b = np.asarray(inputs["mix1_bias"], dtype=np.float32)
    m2w = np.asarray(inputs["mix2_weight"], dtype=np.float32)

    a1 = m1w[:, 0, :]
    c1 = m1w[:, 1, :]
    w2 = m2w[:, :, 0]

    if "nc" not in _cache:
        _cache["nc"] = build_kernel()
    nc = _cache["nc"]

    wq_s = Wq * (1.0 / np.sqrt(D))

    w1l = np.zeros((17, 256), dtype=np.float32)
    w2l = np.zeros((128, 16), dtype=np.float32)
    bcol2 = np.zeros((128, 2), dtype=np.float32)
    for h in range(H):
        half, hl = h // 8, h % 8
        for m in range(MS):
            col = half * 128 + hl * 16 + m
            w1l[h, col] = a1[h, m]
            w1l[16, col] = c1[h, m]
            w2l[hl * 16 + m, half * 8 + hl] = w2[h, m]
            bcol2[hl * 16 + m, half] = m1b[h, m]

    in_maps = []
    for i in range(NCORES):
        sl = slice(i * BL, (i + 1) * BL)
        in_maps.append({
            "x_r": row_emb[sl].reshape(TOK, E),
            "x_c": col_emb[sl].reshape(TOK, E),
            "cost": cost_mat[sl],
            "Wq": wq_s, "Wk": Wk, "Wv": Wv,
            "W1L": w1l, "W2L": w2l, "bcol2": bcol2,
        })
    res = run_bass_kernel_spmd(nc, in_maps, list(range(NCORES)))
    out = np.concatenate([res.results[i]["out"] for i in range(NCORES)],
                         axis=0)
    return out.astype(np.float32)


# revision 32
# speedup vs baseline: 1.6672x; 1.6672x over previous
"""MixedScoreMultiHeadAttention Trainium2 kernel, v2.

Data-parallel over batch: 32 batches -> 8 cores x 4 batches.

Cost-model-driven design (TimelineSim):
 - matmul cost = out-free-size x 0.417ns (fp16 moving); ldweights free
 - ACT/DVE evac cost = free-size x 0.83/1.04 ns + ~0.13-0.37us fixed/op
 - Pool cannot read PSUM; evac of the mix-MLP hidden layer (131072
   rows/core) across ACT+DVE is the bottleneck -> use 2-bank [128,1024]
   PSUM tiles so each evac op moves 1024 rows, and balance ACT/DVE.
 - full-partition single-op projection evacs (zero-padded head slots ride
   along free); one-DMA rhs assembly per batch (HWDGE is 625ns/DMA).

Pipeline per (b): dots (PE) -> d_all fp16 -> DMA-assemble rhs [17, 16384]
(r-major points; row 16 = cost) -> layer1 [17x128 stationary] per half ->
relu evac (bias folded, ACT/DVE alternating) -> layer2 (stationary = data,
8-col moving w2 blockdiag) -> exp evac -> AV with ones-column denominator
-> reciprocal * numerator.

mix2 bias dropped (softmax-invariant); 1/sqrt(D) folded into Wq host-side.
"""
import sys

sys.path.insert(0, "/opt/trn_rl_repo")

import numpy as np
from contextlib import ExitStack

import concourse.bass as bass
import concourse.mybir as mybir
import concourse.tile as tile
from concourse import bacc
from concourse.bass_utils import run_bass_kernel_spmd
from concourse.masks import make_identity

B, R, C, E, H, D, MS = 32, 128, 128, 256, 16, 16, 16
NCORES = 8
BL = B // NCORES  # batches per core: 4
TOK = BL * R      # 512 tokens per core per side
PTS = R * C       # 16384 score points per (b)

FP32 = mybir.dt.float32
FP16 = mybir.dt.float16
AF = mybir.ActivationFunctionType
ALU = mybir.AluOpType


def cp(nc, use_act, out, in_):
    if use_act:
        nc.scalar.copy(out, in_)
    else:
        nc.vector.tensor_copy(out, in_)


def build_kernel():
    nc = bacc.Bacc("TRN2", target_bir_lowering=False, debug=False,
                   num_devices=NCORES)

    x_r = nc.dram_tensor("x_r", [TOK, E], FP32, kind="ExternalInput").ap()
    x_c = nc.dram_tensor("x_c", [TOK, E], FP32, kind="ExternalInput").ap()
    cost = nc.dram_tensor("cost", [BL, R, C], FP32, kind="ExternalInput").ap()
    wq_d = nc.dram_tensor("Wq", [E, E], FP32, kind="ExternalInput").ap()
    wk_d = nc.dram_tensor("Wk", [E, E], FP32, kind="ExternalInput").ap()
    wv_d = nc.dram_tensor("Wv", [E, E], FP32, kind="ExternalInput").ap()
    # layer1 stationary [17, 256]: col (half*128 + (h%8)*16 + m):
    #   row h' = a[h,m] iff h'==h; row 16 = c[h,m]
    w1_d = nc.dram_tensor("W1L", [17, 2 * 128], FP32,
                          kind="ExternalInput").ap()
    # layer2 moving [128, 16]: col (half*8 + j): row hm = w2[half*8+j, m]
    # iff hm == (j*16+m) else 0
    w2_d = nc.dram_tensor("W2L", [128, 16], FP32, kind="ExternalInput").ap()
    # relu bias per (h,m) row: bcol2[hm, half] = b1[half*8 + hm//16, hm%16]
    bc_d = nc.dram_tensor("bcol2", [128, 2], FP32, kind="ExternalInput").ap()
    out_d = nc.dram_tensor("out", [BL, R, H * D], FP32,
                           kind="ExternalOutput").ap()

    with tile.TileContext(nc) as tc, ExitStack() as ctx:
        const_p = ctx.enter_context(tc.tile_pool(name="const", bufs=1))
        inx_p = ctx.enter_context(tc.tile_pool(name="inx", bufs=2))
        qkv_p = ctx.enter_context(tc.tile_pool(name="qkv", bufs=1))
        dall_p = ctx.enter_context(tc.tile_pool(name="dall", bufs=2))
        rhs_p = ctx.enter_context(tc.tile_pool(name="rhs", bufs=2))
        rr_p = ctx.enter_context(tc.tile_pool(name="rr", bufs=3))
        wsb_p = ctx.enter_context(tc.tile_pool(name="wsb", bufs=2))
        fout_p = ctx.enter_context(tc.tile_pool(name="fout", bufs=1))
        small_p = ctx.enter_context(tc.tile_pool(name="small", bufs=2))
        # PSUM: one big [128,1024] 2-bank rotating pool (bufs=3 -> 6 banks,
        # shared by layer1 chunks / dots / AV-psa / setup) + ps2 2x1 bank
        pl1_p = ctx.enter_context(
            tc.tile_pool(name="pl1", bufs=3, space="PSUM"))
        ps2_p = ctx.enter_context(
            tc.tile_pool(name="ps2", bufs=2, space="PSUM"))

        ident = const_p.tile([128, 128], FP32)
        make_identity(nc, ident[:])

        # PE p-state warmup: keep the tensor engine busy from t~0 so the
        # ramp (full clock after 3us of busy) completes before the real work
        warm = const_p.tile([16, 16], FP16)
        nc.gpsimd.memset(warm[:], 0.0)
        wexp = const_p.tile([16, 16], FP32)
        # prefetch the ACT function table (Relu/Exp/Copy set) at t~0 so the
        # 1.3us table load isn't serialized into the first projection evac
        nc.scalar.activation(wexp[0:1, 0:1], warm[0:1, 0:1], AF.Exp)
        psw = pl1_p.tile([128, 1024], FP32, tag="l1")
        for i in range(40):
            nc.tensor.matmul(psw[0:16, (i % 8) * 64:(i % 8) * 64 + 16],
                             warm[:], warm[:])

        # ---- input DMAs: q/k weights and x first (both gate the b0
        # fast-path projections), then everything else
        wlds = {}
        xlds = {}
        for name, dram in (("q", wq_d), ("k", wk_d)):
            w32 = inx_p.tile([128, 2 * E], FP32, tag=f"wload{name}",
                             name=f"wload{name}")
            nc.sync.dma_start(
                w32[:].rearrange("p (s e) -> p s e", s=2),
                dram[:].rearrange("(s p) e -> p s e", p=128))
            wlds[name] = w32
        for name, dram in (("r", x_r), ("c", x_c)):
            xld = inx_p.tile([128, 4 * E], FP32, tag=f"xload{name}",
                             name=f"xload{name}")
            nc.sync.dma_start(xld[:, 0:E], dram[0:128, :])
            xlds[name] = xld
        for name, dram in (("r", x_r), ("c", x_c)):
            for t in range(1, 4):
                nc.sync.dma_start(
                    xlds[name][:, t * E:(t + 1) * E],
                    dram[t * 128:(t + 1) * 128, :])
        y32 = inx_p.tile([128, BL * C], FP32, tag="y32")
        nc.sync.dma_start(
            y32[:].rearrange("p (b c) -> p b c", b=BL),
            cost[:].rearrange("b r c -> r b c"))
        w32v = inx_p.tile([128, 2 * E], FP32, tag="wloadv", name="wloadv")
        nc.sync.dma_start(
            w32v[:].rearrange("p (s e) -> p s e", s=2),
            wv_d[:].rearrange("(s p) e -> p s e", p=128))
        wlds["v"] = w32v
        w1f = inx_p.tile([17, 2 * 128], FP32, tag="w1f")
        nc.sync.dma_start(w1f[:], w1_d[:])
        w2f = inx_p.tile([128, 16], FP32, tag="w2f")
        nc.sync.dma_start(w2f[:], w2_d[:])
        bcol2 = const_p.tile([128, 2], FP32)
        nc.sync.dma_start(bcol2[:], bc_d[:])

        # ---- x PE-transpose -> xT fp16 [2 e-halves][128, TOK]
        xT = {}
        for name in ("r", "c"):
            xt0 = const_p.tile([128, TOK], FP16, tag=f"xT{name}0",
                               name=f"xT{name}0")
            xt1 = const_p.tile([128, TOK], FP16, tag=f"xT{name}1",
                               name=f"xT{name}1")
            xT[name] = [xt0, xt1]
            xld = xlds[name]
            pst = pl1_p.tile([128, 1024], FP32, tag="l1")
            for t in range(4):
                for eh in range(2):
                    nc.tensor.transpose(
                        pst[:, eh * 512 + t * 128:eh * 512 + t * 128 + 128],
                        xld[:, t * E + eh * 128:t * E + eh * 128 + 128],
                        ident[:])
            for eh in range(2):
                cp(nc, (name == "r") ^ (eh == 0),
                   xT[name][eh][:], pst[:, eh * 512:eh * 512 + 512])

        # ---- QKV weights fp16 (q/k padded on-chip: head h -> 32-col slot)
        wt16 = {}
        for name in ("q", "k", "v"):
            w32 = wlds[name]
            halves = []
            for eh in range(2):
                ncols = E if name == "v" else 2 * E
                w16 = const_p.tile([128, ncols], FP16, tag=f"w16{name}{eh}",
                                   name=f"w16{name}{eh}")
                w32e = w32[:, eh * E:(eh + 1) * E]
                if name == "v":
                    nc.gpsimd.tensor_copy(w16[:], w32e)
                else:
                    nc.gpsimd.memset(w16[:], 0.0)
                    w16v = w16[:].rearrange("p (h x) -> p h x", h=H)
                    w32v = w32e.rearrange("p (h x) -> p h x", h=H)
                    nc.gpsimd.tensor_copy(w16v[:, :, 0:D], w32v[:])
                halves.append(w16)
            wt16[name] = halves

        # ---- b0 fast path: project only b0's 128 tokens per side so the
        # rhs(b0) DMA chain starts ~10us earlier than the full projections
        qkT0 = {}
        qk30 = {}
        for proj in ("q", "k"):
            src0 = xT["r"] if proj == "q" else xT["c"]
            ps = pl1_p.tile([128, 1024], FP32, tag="l1")
            for mh in range(4):
                for eh in range(2):
                    nc.tensor.matmul(
                        ps[:, mh * 128:(mh + 1) * 128],
                        wt16[proj][eh][:, mh * 128:(mh + 1) * 128],
                        src0[eh][:, 0:128], start=(eh == 0), stop=(eh == 1))
            qt0 = const_p.tile([128, 512], FP16, tag=f"qkT0{proj}",
                               name=f"qkT0{proj}")
            qkT0[proj] = qt0
            cp(nc, proj == "q", qt0[:], ps[:, 0:512])
            qk30[proj] = {}
            for cls in (1, 2, 3):
                q30 = const_p.tile([16, 512], FP16, tag=f"qk30{proj}{cls}",
                                   name=f"qk30{proj}{cls}")
                qk30[proj][cls] = q30
                cp(nc, cls == 2, q30[:], qt0[cls * 32:cls * 32 + 16, :])

        def qk_slice(proj, h, b):
            # [16, 128] operand, always at base partition 0
            mh, cls = h // 4, h % 4
            if b == 0:
                lo = mh * 128
                if cls == 0:
                    return qkT0[proj][0:16, lo:lo + 128]
                return qk30[proj][cls][0:16, lo:lo + 128]
            lo = mh * TOK + b * 128
            if cls == 0:
                return qkT[proj][0:16, lo:lo + 128]
            return qk3[proj][cls][0:16, lo:lo + 128]

        # ---- small converts (needed from dots/layer1 on; emitted after the
        # projection evacs so they don't delay the xT/proj critical path)
        y16 = const_p.tile([128, BL * C], FP16)
        nc.gpsimd.tensor_copy(y16[:], y32[:])
        w1l = const_p.tile([17, 2 * 128], FP16)
        nc.gpsimd.tensor_copy(w1l[:], w1f[:])
        w2l = const_p.tile([128, 16], FP16)
        nc.gpsimd.tensor_copy(w2l[:], w2f[:])

        # ---- v -> vhat_all [128 c, (b, h, 17)] fp32, ones in col 16
        # (emitted after dots(0) below, shadowed by the rhs(0) DMA chain)
        vhat = qkv_p.tile([128, BL * H * 17], FP32)
        vh4 = vhat[:].rearrange("p (b h x) -> p b h x", b=BL, h=H)

        def emit_v():
            nc.gpsimd.memset(vh4[:, :, :, 16:17], 1.0)
            psv = pl1_p.tile([128, 1024], FP32, tag="l1")
            for b4 in range(BL):
                for eh in range(2):
                    nc.tensor.matmul(
                        psv[:, b4 * 256:b4 * 256 + 256],
                        xT["c"][eh][:, b4 * 128:(b4 + 1) * 128],
                        wt16["v"][eh][:], start=(eh == 0), stop=(eh == 1))
            nc.scalar.copy(
                vh4[:, :, :, 0:16],
                psv[:].rearrange("p (b h x) -> p b h x", b=BL, h=H))

        # ---- per-b pipeline, software-pipelined:
        #  - dots/rhs for b+1 emitted early inside b's first half
        #  - layer2(ck) emitted after layer1(ck+1) (lag-1) so the PE never
        #    blocks at an ldweights waiting for the relu evac
        #  - AV/normalize for a half deferred ~2 chunks into the next half
        fouts = [fout_p.tile([128, H * D], FP32, tag=f"fo{b}",
                             name=f"fo{b}") for b in range(BL)]

        def emit_dots(b):
            # dots: 4 rounds of 4 heads -> d_all [128 r, (h, c)] fp16,
            # then per-head DMAs into rhs [17, PTS] (r-major points)
            rhs = rhs_p.tile([17, PTS], FP16, tag="rhs")
            nc.sync.dma_start(rhs[16:17, :],
                              y16[:, b * C:(b + 1) * C])
            for g8 in range(2):
                d_g = dall_p.tile([128, 8 * C], FP16, tag=f"dall{g8}",
                                  name=f"dall{g8}")
                psd = pl1_p.tile([128, 1024], FP32, tag="l1")
                for hh in range(8):
                    h = g8 * 8 + hh
                    nc.tensor.matmul(
                        psd[:, hh * 128:(hh + 1) * 128],
                        qk_slice("q", h, b), qk_slice("k", h, b))
                cp(nc, g8 % 2 == 0, d_g[:], psd[:])
                for hh in range(8):
                    h = g8 * 8 + hh
                    # split across HWDGE (sync/SP) and SWDGE (gpsimd/Pool)
                    # so the reshuffle DMAs don't serialize on one DGE
                    eng = nc.sync if hh < 5 else nc.gpsimd
                    eng.dma_start(rhs[h:h + 1, :],
                                  d_g[:, hh * C:(hh + 1) * C])
            return rhs

        def emit_l2(ps2, half, ck, rr):
            for s in range(8):
                rloc = (ck % 8) * 8 + s
                nc.tensor.matmul(
                    ps2[:, rloc * 8:rloc * 8 + 8],
                    rr[:, s * 128:(s + 1) * 128],
                    w2l[:, half * 8:(half + 1) * 8])

        def emit_av(b, half, wsb):
            # AV with ones-column denominator, then reciprocal * numerator
            psa = pl1_p.tile([128, 1024], FP32, tag="l1")
            wsb3 = wsb[:].rearrange("p (r h) -> p r h", r=R)
            for hl in range(8):
                h = half * 8 + hl
                nc.tensor.matmul(
                    psa[:, hl * 17:(hl + 1) * 17],
                    wsb3[:, :, hl],
                    vh4[:, b, h, :])  # cols 0:136 of the 1024-wide tile
            rec = small_p.tile([128, 8], FP32, tag="rec")
            psa3 = psa[:, 0:136].rearrange("p (x y) -> p x y", x=8)
            nc.vector.reciprocal(rec[:], psa3[:, :, 16])
            nc.vector.tensor_tensor(
                fouts[b][:, half * 128:(half + 1) * 128].rearrange(
                    "p (x y) -> p x y", x=8),
                psa3[:, :, 0:16],
                rec[:].unsqueeze(-1).broadcast_to([128, 8, 16]),
                ALU.mult)
            if half == 1:
                nc.sync.dma_start(out_d[b], fouts[b][:])

        rhs = emit_dots(0)
        # ---- projections q/k: full-partition quad tiles [128, 2048] fp16
        # (head h lives at partition slot (h%4)*32, mh = h//4 selects the
        #  512-col token range; zero rows ride along for free)
        qkT = {}
        qk3 = {}
        for proj in ("q", "k"):
            qt = const_p.tile([128, 4 * TOK], FP16, tag=f"qkT{proj}",
                              name=f"qkT{proj}")
            qkT[proj] = qt
            src2 = xT["r"] if proj == "q" else xT["c"]
            for mp in range(2):  # mh pairs
                ps = pl1_p.tile([128, 1024], FP32, tag="l1")
                for mi in range(2):
                    mh = mp * 2 + mi
                    for eh in range(2):
                        nc.tensor.matmul(
                            ps[:, mi * 512:(mi + 1) * 512],
                            wt16[proj][eh][:, mh * 128:(mh + 1) * 128],
                            src2[eh][:], start=(eh == 0), stop=(eh == 1))
                cp(nc, mp % 2 == 0,
                   qt[:, mp * 1024:(mp + 1) * 1024], ps[:])
            # non-base-0 matmul operands crash walrus-lowered HW here --
            # re-home head slots 1..3 to base-0 tiles (cheap SBUF fp16)
            qk3[proj] = {}
            for cls in (1, 2, 3):
                q3 = const_p.tile([16, 4 * TOK], FP16,
                                  tag=f"qk3{proj}{cls}",
                                  name=f"qk3{proj}{cls}")
                qk3[proj][cls] = q3
                cp(nc, cls == 2, q3[:], qt[cls * 32:cls * 32 + 16, :])

        emit_v()
        pend_l2 = None    # (ps2grp, half, ck, rr)
        pend_exp = None   # (wsb, grp, ps2grp)
        pend_av = None    # (b, half, wsb)
        next_rhs = None
        wsb = None
        for b in range(BL):
            for half in range(2):
                for ck in range(16):
                    if ck == 0:
                        wsb_prev, wsb = wsb, wsb_p.tile(
                            [128, 8 * C], FP32, tag="wsb")
                        ps2 = ps2_p.tile([128, 512], FP32, tag="ps2")
                    if ck == 8:
                        ps2 = ps2_p.tile([128, 512], FP32, tag="ps2")
                    pl = pl1_p.tile([128, 1024], FP32, tag="l1")
                    for s2 in range(2):
                        nc.tensor.matmul(
                            pl[:, s2 * 512:(s2 + 1) * 512],
                            w1l[:, half * 128:(half + 1) * 128],
                            rhs[:, ck * 1024 + s2 * 512:
                                ck * 1024 + (s2 + 1) * 512])
                    rr = rr_p.tile([128, 1024], FP16, tag="rr")
                    if ck % 2 == 0:
                        nc.scalar.activation(
                            rr[:], pl[:], AF.Relu,
                            bias=bcol2[:, half:half + 1])
                    else:
                        nc.vector.tensor_scalar(
                            rr[:], pl[:], bcol2[:, half:half + 1],
                            0.0, ALU.add, ALU.max)
                    if pend_l2 is not None:
                        emit_l2(*pend_l2)
                    pend_l2 = (ps2, half, ck, rr)
                    if ck == 0 and pend_exp is not None:
                        # grp-B exp of the previous half (l2 flushed above)
                        pwsb, pgrp, pps2 = pend_exp
                        nc.scalar.activation(
                            pwsb[:, pgrp * 512:(pgrp + 1) * 512],
                            pps2[:], AF.Exp)
                        pend_exp = None
                    if ck == 2:
                        if pend_av is not None:
                            emit_av(*pend_av)
                            pend_av = None
                        if half == 0 and b + 1 < BL:
                            next_rhs = emit_dots(b + 1)
                    if ck == 8:
                        # grp-A l2 was flushed above; exp it now so grp-A's
                        # ps2 bank is free again before the next half
                        pa = pend_l2  # == (ps2B, half, 8, rr) not grp A
                        nc.scalar.activation(
                            wsb[:, 0:512], ps2_grpa[:], AF.Exp)
                    if ck == 0:
                        ps2_grpa = ps2
                pend_exp = (wsb, 1, ps2)
                pend_av = (b, half, wsb)
            rhs = next_rhs
        if pend_l2 is not None:
            emit_l2(*pend_l2)
            pend_l2 = None
        pwsb, pgrp, pps2 = pend_exp
        nc.scalar.activation(
            pwsb[:, pgrp * 512:(pgrp + 1) * 512], pps2[:], AF.Exp)
        emit_av(*pend_av)

    nc.compile()
    return nc


_cache = {}


def kernel(**inputs):
    row_emb = np.asarray(inputs["row_emb"], dtype=np.float32)
    col_emb = np.asarray(inputs["col_emb"], dtype=np.float32)
    cost_mat = np.asarray(inputs["cost_mat"], dtype=np.float32)
    Wq = np.asarray(inputs["Wq"], dtype=np.float32)
    Wk = np.asarray(inputs["Wk"], dtype=np.float32)
    Wv = np.asarray(inputs["Wv"], dtype=np.float32)
    m1w = np.asarray(inputs["mix1_weight"], dtype=np.float32)
    m1b = np.asarray(inputs["mix1_bias"], dtype=np.float32)
    m2w = np.asarray(inputs["mix2_weight"], dtype=np.float32)

    a1 = m1w[:, 0, :]
    c1 = m1w[:, 1, :]
    w2 = m2w[:, :, 0]

    if "nc" not in _cache:
        _cache["nc"] = build_kernel()
    nc = _cache["nc"]

    wq_s = Wq * (1.0 / np.sqrt(D))

    w1l = np.zeros((17, 256), dtype=np.float32)
    w2l = np.zeros((128, 16), dtype=np.float32)
    bcol2 = np.zeros((128, 2), dtype=np.float32)
    for h in range(H):
        half, hl = h // 8, h % 8
        for m in range(MS):
            col = half * 128 + hl * 16 + m
            w1l[h, col] = a1[h, m]
            w1l[16, col] = c1[h, m]
            w2l[hl * 16 + m, half * 8 + hl] = w2[h, m]
            bcol2[hl * 16 + m, half] = m1b[h, m]

    in_maps = []
    for i in range(NCORES):
        sl = slice(i * BL, (i + 1) * BL)
        in_maps.append({
            "x_r": row_emb[sl].reshape(TOK, E),
            "x_c": col_emb[sl].reshape(TOK, E),
            "cost": cost_mat[sl],
            "Wq": wq_s, "Wk": Wk, "Wv": Wv,
            "W1L": w1l, "W2L": w2l, "bcol2": bcol2,
        })
    res = run_bass_kernel_spmd(nc, in_maps, list(range(NCORES)))
    out = np.concatenate([res.results[i]["out"] for i in range(NCORES)],
                         axis=0)
    return out.astype(np.float32)
